# revision 49
# baseline (speedup 1.0000x reference)
"""Trainium2 Bass kernel for the DiffRenderer problem.

Math refactor (validated against the jax reference):
  The renderer's per-point MLP input collapses to
      a[b, pix, d, h] = depth[b, d] * g[b, pix, h] + e[b, h]
  with
      g[b] = Q @ V_b,  V_b = M_b^T @ W1[:3],  M_b = s_obj * R_obj
      e[b] = c_b @ W1[:3] + z_shape[b] @ W1[3:] + b1
      depth[b, d] = zs[d] * s_inv[b] + bb_depth[b]
  Layer 1 + relu:  h' = max(depth*g, -e)   (== relu(a) - e)
  Layer 2:         s  = W2 . h' + (W2 . e + b2)   (bias folded into tanh)
  sdf = tanh(s); then the zero-crossing depth extraction (s1/s2/d1r on
  device, final interpolation on host in fp32 mirroring the reference).

Sharding: 8 cores = 4 batches x 2 pixel-halves (2048 pixels/core, 64 depths).

Per-core device pipeline (all phases overlap via the Tile scheduler):
  PE:  p-state warmup matmuls; g = Q @ V (float32r); 64 sliding one-hot
       layer-2 matmuls with W2 stationary [128,64] so psum row d
       accumulates s[d, :] directly (float32r = 1 cycle/row);
       16 transposes [64,128] -> [128,64]
  DVE: h' = max(depth_d * g, -e) for pixels 0..1279 (tensor_scalar
       mult+max, 2x perf mode); zero-crossing postprocess (pos/zc/
       s1-product/all reduces)
  ACT: h' = relu(depth_d * g + e) for pixels 1280..2047 (activation with
       per-partition scale/bias); psum->sbuf copies; tanh (+bias fold)
  GPS: input DMA queue; s2/d1 tail products (tensor_tensor)
The final occ/depth interpolation runs on the host in fp32 from the
packed s1/s2/d1r outputs, mirroring the reference arithmetic exactly.

Measured phase budget (core 0): ~14us startup (fixed preamble ~7 +
DMA roundtrip ~2.3 + g phase), ~59.4us d-loop (PE-bound, 928 ns/iter,
all three engines 95-99% busy), ~8.4us tail (DVE reduce-bound),
~9.5us fixed framework teardown (249 unconditional semaphore resets).

Precision: float32r (FP22 in the PE) for g and layer-2, fp32 elsewhere;
  bf16 was measured to flip sdf signs in the randn-weight regime and is
  deliberately NOT used.
"""

import os
import sys

import numpy as np

for _p in ("/opt/trn_rl_repo", "/root/.axon_site/_ro/trn_rl_repo"):
    if os.path.isdir(_p) and _p not in sys.path:
        sys.path.append(_p)

from contextlib import ExitStack

from concourse import bacc, bass, masks, mybir, tile
from concourse.bass_utils import run_bass_kernel_spmd

F32 = mybir.dt.float32
F32R = mybir.dt.float32r
ALU = mybir.AluOpType
ACTF = mybir.ActivationFunctionType

IMG = 64
D = 64
HID = 128
BS = 4
NCORES = 8
PIX = IMG * IMG          # 4096 pixels per batch
PPC = PIX // 2           # 2048 pixels per core
NT = PPC // 128          # 16 pixel tiles per core
K63 = 63                 # depth pairs per tile

# engine split of the 2048 h' columns per depth step (GpSimd tensor_scalar
# measured ~17.6 ns/col on hw - software Q7 path - so the main loop stays on
# DVE+ACT only; GpSimd gets tail tensor_tensor work at ~2.3 ns/col instead).
# Psum chunks are <=512 wide (one bank) and >=256 (f32r 1 cycle/row needs
# moving size >= 256); chunks 0-2 are max-form (bias0), 3-4 relu (bias1).
DVE_N = 1280             # pixels 0..1279 on DVE (max-form)
ACT_N = 768              # pixels 1280..2047 on ACT (relu-form)
CHUNKS = [(0, 512), (512, 512), (1024, 256), (1280, 512), (1792, 256)]
CHUNK_TILES = [(0, 4), (4, 8), (8, 10), (10, 14), (14, 16)]

# consts tensor columns
C_NEGE = 0
C_EPOS = 1
C_B0 = 2
C_B1 = 3
C_DEPTH = 4              # cols 4..68 = depth[64]
C_W = 4 + D

_PROGRAM = None


def build_program():
    nc = bacc.Bacc(None, target_bir_lowering=False)
    vb = nc.declare_dram_parameter("vb", [3, HID], F32R, isOutput=False)
    qs = nc.declare_dram_parameter("qs", [3, PPC], F32R, isOutput=False)
    consts = nc.declare_dram_parameter("consts", [128, C_W], F32, isOutput=False)
    w2w = nc.declare_dram_parameter("w2w", [HID, 127], F32R, isOutput=False)
    out_o = nc.declare_dram_parameter("out", [128, 3 * NT], F32, isOutput=True)

    with tile.TileContext(nc) as tc, ExitStack() as ctx:
        const = ctx.enter_context(tc.tile_pool(name="const", bufs=1))
        gpool = ctx.enter_context(tc.tile_pool(name="g", bufs=1))
        hpool = ctx.enter_context(tc.tile_pool(name="hp", bufs=6))
        spool = ctx.enter_context(tc.tile_pool(name="s", bufs=1))
        post = ctx.enter_context(tc.tile_pool(name="post", bufs=1))
        pss = ctx.enter_context(tc.tile_pool(name="pss", bufs=1, space="PSUM"))
        pst = ctx.enter_context(tc.tile_pool(name="pst", bufs=2, space="PSUM"))

        # ---- input loads: vb at the head of the gpsimd queue (it can
        # enqueue ~0.4us after the preamble vs ~6.8us on sync, whose first
        # slot is delayed by a preamble drain); qs alone on sync ----
        t_vb = const.tile([3, HID], F32R, name="t_vb", tag="t_vb")
        nc.gpsimd.dma_start(t_vb[:], vb[:])
        t_qs = const.tile([3, PPC], F32R, name="t_qs", tag="t_qs")
        nc.sync.dma_start(t_qs[:], qs[:])
        # PE p-state warmup: dummy matmuls on a DVE-memset scratch tile while
        # the input DMAs are in flight (the PE clock ramps 0.65 -> 2.4 GHz
        # after ~3us of continuous work; without this the g matmuls and the
        # first loop iterations run up to ~2.4x slow)
        warm = const.tile([128, 256], F32, name="warm", tag="warm")
        nc.vector.memset(warm[:], 0.0)
        pwarm = pss.tile([64, 256], F32, name="pwarm", tag="pwarm")
        for _ in range(3):
            nc.tensor.matmul(
                pwarm[:], warm[:, 0:64], warm[:], start=True, stop=True)

        t_c = const.tile([128, C_W], F32, name="t_c", tag="t_c")
        nc.gpsimd.dma_start(t_c[:], consts[:])
        # sliding-window one-hot W2 (host-built): col 63 holds W2, rest
        # zero; slice [:, 63-d : 127-d] puts W2 in stationary column d ->
        # matmul accumulates s[d, :] into psum partition d.
        w2win = const.tile([HID, 127], F32R, name="w2win", tag="w2win")
        nc.gpsimd.dma_start(w2win[:], w2w[:])

        # identity for the tail transposes (gpsimd ops, needed only late)
        ident = const.tile([64, 64], F32)
        masks.make_identity(nc, ident[:])

        t_nege = t_c[:, C_NEGE:C_NEGE + 1]
        t_epos = t_c[:, C_EPOS:C_EPOS + 1]
        t_b0 = t_c[:, C_B0:C_B0 + 1]
        t_b1 = t_c[:, C_B1:C_B1 + 1]
        t_depth = t_c[:, C_DEPTH:C_DEPTH + D]

        # zm1[k] = depth[k] - 100 (first-crossing depth extraction); the d1
        # product reads it through a stride-0 broadcast AP over the tile axis
        zm1 = const.tile([128, K63], F32, name="zm1", tag="zm1")
        nc.vector.tensor_scalar(zm1[:], t_depth[:, 0:K63], -100.0, None, op0=ALU.add)

        # ---- g = Q @ V  (float32r matmul; one-time) ----
        # psum slots shared (by tag) with the layer-2 chunk tiles: the g
        # phase finishes before the d-loop's accumulators are first used.
        g_sb = gpool.tile([HID, PPC], F32)
        for k in range(4):
            pg = pss.tile([HID, 512], F32, name=f"pg{k}", tag=f"pss{k}")
            nc.tensor.matmul(
                pg[:], t_vb[:], t_qs[:, k * 512:(k + 1) * 512],
                start=True, stop=True,
            )
            dst = g_sb[:, k * 512:(k + 1) * 512]
            if k % 2 == 0:
                nc.vector.tensor_copy(dst, pg[:])
            else:
                nc.scalar.copy(dst, pg[:])

        # ---- main d-loop: h' then layer-2 (accumulating masked matmuls) ----
        ps_chunks = [
            pss.tile([D, n], F32, name=f"ps_chunk{k}", tag=f"pss{k}")
            for k, (off, n) in enumerate(CHUNKS)
        ]
        for d in range(D):
            hp = hpool.tile([HID, PPC], F32R, tag="hp")
            dep_d = t_depth[:, d:d + 1]
            if d == 0:
                # split along the g-copy chunks so the first layer-2
                # matmuls start before the last g copy lands
                for o0, o1 in ((0, 512), (512, 1024), (1024, DVE_N)):
                    nc.vector.tensor_scalar(
                        hp[:, o0:o1], g_sb[:, o0:o1],
                        dep_d, t_nege, op0=ALU.mult, op1=ALU.max,
                    )
            else:
                nc.vector.tensor_scalar(
                    hp[:, 0:DVE_N], g_sb[:, 0:DVE_N],
                    dep_d, t_nege, op0=ALU.mult, op1=ALU.max,
                )
            nc.scalar.activation(
                hp[:, DVE_N:PPC], g_sb[:, DVE_N:PPC], ACTF.Relu,
                bias=t_epos, scale=dep_d,
            )
            w2slice = w2win[:, 63 - d:127 - d]
            for k, (off, n) in enumerate(CHUNKS):
                nc.tensor.matmul(
                    ps_chunks[k][:], w2slice, hp[:, off:off + n],
                    start=(d == 0), stop=(d == D - 1),
                    skip_group_check=True,
                )

        # ---- tail: per-chunk copy, transpose, tanh; then postprocess ----
        s_sb = spool.tile([D, PPC], F32)
        sdf = post.tile([128, NT * D], F32)
        pos = post.tile([128, NT * D], F32)
        zc = post.tile([128, NT * K63], F32)
        out_sb = post.tile([128, 3 * NT], F32)
        sdf3 = sdf[:].rearrange("p (t d) -> p t d", d=D)
        pos3 = pos[:].rearrange("p (t d) -> p t d", d=D)
        zc3 = zc[:].rearrange("p (t k) -> p t k", k=K63)
        zm1_1 = zm1[:].rearrange("p (o k) -> p o k", o=1)
        for k, (off, n) in enumerate(CHUNKS):
            if k % 2 == 0:
                nc.scalar.copy(s_sb[:, off:off + n], ps_chunks[k][:])
            else:
                nc.vector.tensor_copy(s_sb[:, off:off + n], ps_chunks[k][:])
            t0, t1 = CHUNK_TILES[k]
            # transposes share one psum tile per chunk -> single batched tanh
            # (chunks 0-2 are all max-form tiles, chunks 3-4 relu-form, so
            # the per-partition tanh bias is uniform within a chunk)
            pt = pst.tile([128, (t1 - t0) * D], F32, name=f"pt{k}", tag="pt", bufs=2)
            for j, i in enumerate(range(t0, t1)):
                nc.tensor.transpose(
                    pt[:, j * D:(j + 1) * D], s_sb[:, i * 128:(i + 1) * 128], ident[:])
            b_ap = t_b0 if k < 3 else t_b1
            nc.scalar.activation(
                sdf[:, t0 * D:t1 * D], pt[:, 0:(t1 - t0) * D], ACTF.Tanh,
                bias=b_ap, scale=1.0,
            )

        # postprocess in 2 tile groups (pipelined); each group's reduces
        # write disjoint columns of the packed output tile:
        #   out_sb[:, 0:16] = s1, [:, 16:32] = s2, [:, 32:48] = d1r
        for t0, t1 in ((0, 8), (8, 16)):
            ts_ = slice(t0, t1)
            nc.vector.tensor_scalar(
                pos[:, t0 * D:t1 * D],
                sdf[:, t0 * D:t1 * D], 0.0, None, op0=ALU.is_gt)
            nc.vector.scalar_tensor_tensor(
                zc3[:, ts_, :], pos3[:, ts_, 1:D], 0.5, pos3[:, ts_, 0:K63],
                op0=ALU.is_lt, op1=ALU.mult,
            )
            nw = (t1 - t0) * K63
            # s2/d1 products on GpSimd (run in parallel); reduces are DVE-only
            for qi, (in1, red_op, oc, eng_m) in enumerate((
                (sdf3[:, ts_, 0:K63], ALU.add, 0, nc.vector),
                (sdf3[:, ts_, 1:D], ALU.add, 1, nc.gpsimd),
                (zm1_1.broadcast_to([128, t1 - t0, K63]), ALU.min, 2, nc.gpsimd),
            )):
                tmp = post.tile([128, 8 * K63], F32, tag=f"ppt{qi}", bufs=2)
                tmp3 = tmp[:, 0:nw].rearrange("p (t k) -> p t k", k=K63)
                eng_m.tensor_tensor(tmp3, zc3[:, ts_, :], in1, op=ALU.mult)
                nc.vector.tensor_reduce(
                    out_sb[:, oc * NT + t0:oc * NT + t1], tmp3,
                    axis=mybir.AxisListType.X, op=red_op)

        # single output DMA on the sync ring (measured lowest completion
        # latency; the scalar HWDGE ring and a dual-ring split both added
        # ~0.6-0.7us to the reduce->teardown gap)
        nc.sync.dma_start(out_o[:], out_sb[:])

    nc.finalize()
    return nc


def host_prep(z_shape, z_extr, W1, b1, W2, b2):
    """Per-core input maps. All small math mirrors the reference in
    float64 (deviations ~1e-7, far inside the sdf sign margins)."""
    f32 = np.float32
    z_shape = np.asarray(z_shape, f32)
    z_extr = np.asarray(z_extr, f32)
    W1 = np.asarray(W1, f32)
    b1 = np.asarray(b1, f32)
    W2 = np.asarray(W2, f32)
    b2 = np.asarray(b2, f32)

    f = 70.0 * (IMG / 64.0)
    cc = IMG / 2.0 - 0.5
    Km = np.array([[f, 0, cc], [0, f, cc], [0, 0, 1]], np.float64)
    K_inv = np.linalg.inv(Km)
    t = np.array([0.0, 0.0, 2.5])

    # mirror the reference's f32 double-reciprocal
    s_obj32 = (1.0 / z_extr[:, 0]).astype(f32)
    s_inv32 = (1.0 / s_obj32).astype(f32)
    s_obj = s_obj32.astype(np.float64)
    s_inv = s_inv32.astype(np.float64)
    t_obj = z_extr[:, 1:4].astype(np.float64)
    alpha = z_extr[:, 4].astype(np.float64)

    a = np.pi * alpha
    ca, sa = np.cos(a), np.sin(a)
    R_obj = np.zeros((BS, 3, 3))
    R_obj[:, 0, 0] = ca
    R_obj[:, 0, 1] = -sa
    R_obj[:, 1, 0] = sa
    R_obj[:, 1, 1] = ca
    R_obj[:, 2, 2] = 1.0

    corners = np.array(
        [[1, 1, 1], [1, 1, -1], [1, -1, 1], [1, -1, -1],
         [-1, 1, 1], [-1, 1, -1], [-1, -1, 1], [-1, -1, -1], [0, 0, 0]],
        np.float64,
    )
    R_obj_inv = np.linalg.inv(R_obj)
    # z-component of K @ (R_t^-1 (R_obj_inv (s_inv * corner) + t_obj) + t)
    zcr = np.einsum("bij,aj->bai", R_obj_inv, corners)[:, :, 2] * s_inv[:, None]
    bb_depth = zcr.mean(axis=1) + t_obj[:, 2] + 2.5      # (BS,)

    zs = np.linspace(-1.0, 1.0, D)
    depth_bd = (zs[None, :] * s_inv[:, None] + bb_depth[:, None]).astype(f32)

    M = s_obj[:, None, None] * R_obj
    c_b = np.einsum("bij,bj->bi", M, -(t[None, :] + t_obj))
    V = np.einsum("bij,ih->bjh", M, W1[:3].astype(np.float64))   # (BS,3,H)
    e = (
        np.einsum("bi,ih->bh", c_b, W1[:3].astype(np.float64))
        + z_shape.astype(np.float64) @ W1[3:].astype(np.float64)
        + b1.astype(np.float64)
    )
    e32 = e.astype(f32)
    s0 = (e32.astype(np.float64) @ W2.astype(np.float64) + b2.astype(np.float64))
    s0 = s0.astype(f32)                                  # (BS,1)

    xs = np.linspace(0.0, IMG - 1.0, IMG)
    Xg, Yg = np.meshgrid(xs, xs)
    p3 = np.stack([Xg.reshape(-1), Yg.reshape(-1), np.ones(PIX)], -1)
    q = p3 @ K_inv.T                                     # (PIX, 3)

    w2win_host = np.zeros((HID, 127), f32)
    w2win_host[:, 63] = W2[:, 0]
    in_maps = []
    lam_c = []
    for c in range(NCORES):
        b, half = c // 2, c % 2
        qs_c = q[half * PPC:(half + 1) * PPC].T.astype(f32)        # (3, PPC)
        consts = np.zeros((128, C_W), f32)
        consts[:, C_NEGE] = -e32[b]
        consts[:, C_EPOS] = e32[b]
        consts[:, C_B0] = s0[b, 0]
        consts[:, C_B1] = b2[0]
        consts[:, C_DEPTH:C_DEPTH + D] = depth_bd[b][None, :]
        in_maps.append({
            "vb": np.ascontiguousarray(V[b].astype(f32)),
            "qs": np.ascontiguousarray(qs_c),
            "consts": consts,
            "w2w": w2win_host,
        })
        lam_c.append(np.float32(depth_bd[b][1] - depth_bd[b][0]))
    return in_maps, lam_c


def _assemble(results, lam_c):
    """Unpack s1/s2/d1r per core and run the final interpolation on host in
    fp32, mirroring the reference arithmetic."""
    f32 = np.float32
    dp_full = np.zeros((BS, PIX), f32)
    occ_full = np.zeros((BS, PIX), f32)
    eps = f32(1e-6)
    hundred = f32(100.0)
    for c in range(NCORES):
        b, half = c // 2, c % 2
        out = np.asarray(results[c]["out"])               # [128, 48]
        s1 = out[:, 0:NT]
        s2 = out[:, NT:2 * NT]
        d1r = out[:, 2 * NT:3 * NT]
        occ = (d1r <= f32(-50.0)).astype(f32)
        d1 = (d1r + hundred).astype(f32)
        den = (s2 - s1 - eps).astype(f32)
        dp = (occ * (d1 - s1 / den * lam_c[c])).astype(f32)
        sl = slice(half * PPC, (half + 1) * PPC)
        dp_full[b, sl] = dp.T.ravel()
        occ_full[b, sl] = occ.T.ravel()
    return (
        dp_full.reshape(BS, IMG, IMG, 1),
        occ_full.reshape(BS, IMG, IMG, 1),
    )


def get_program():
    global _PROGRAM
    if _PROGRAM is None:
        _PROGRAM = build_program()
    return _PROGRAM


def kernel(z_shape, z_extr, W1, b1, W2, b2, **run_kwargs):
    nc = get_program()
    in_maps, lam_c = host_prep(z_shape, z_extr, W1, b1, W2, b2)
    res = run_bass_kernel_spmd(nc, in_maps, core_ids=list(range(NCORES)), **run_kwargs)
    out = _assemble(res.results, lam_c)
    if run_kwargs:
        return out, res
    return out


# revision 52
# speedup vs baseline: 1.0052x; 1.0052x over previous
"""Trainium2 Bass kernel for the DiffRenderer problem.

Math refactor (validated against the jax reference):
  The renderer's per-point MLP input collapses to
      a[b, pix, d, h] = depth[b, d] * g[b, pix, h] + e[b, h]
  with
      g[b] = Q @ V_b,  V_b = M_b^T @ W1[:3],  M_b = s_obj * R_obj
      e[b] = c_b @ W1[:3] + z_shape[b] @ W1[3:] + b1
      depth[b, d] = zs[d] * s_inv[b] + bb_depth[b]
  Layer 1 + relu:  h' = max(depth*g, -e)   (== relu(a) - e)
  Layer 2:         s  = W2 . h' + (W2 . e + b2)   (bias folded into tanh)
  sdf = tanh(s); then the zero-crossing depth extraction (s1/s2/d1r on
  device, final interpolation on host in fp32 mirroring the reference).

Sharding: 8 cores = 4 batches x 2 pixel-halves (2048 pixels/core, 64 depths).

Per-core device pipeline (all phases overlap via the Tile scheduler):
  PE:  p-state warmup matmuls; g = Q @ V (float32r); 64 sliding one-hot
       layer-2 matmuls with W2 stationary [128,64] so psum row d
       accumulates s[d, :] directly (float32r = 1 cycle/row);
       16 transposes [64,128] -> [128,64]
  DVE: h' = max(depth_d * g, -e) for pixels 0..1279 (tensor_scalar
       mult+max, 2x perf mode); zero-crossing postprocess (pos/zc/
       s1-product/all reduces)
  ACT: h' = relu(depth_d * g + e) for pixels 1280..2047 (activation with
       per-partition scale/bias); psum->sbuf copies; tanh (+bias fold)
  GPS: input DMA queue; s2/d1 tail products (tensor_tensor)
The final occ/depth interpolation runs on the host in fp32 from the
packed s1/s2/d1r outputs, mirroring the reference arithmetic exactly.

Measured phase budget (core 0): ~14us startup (fixed preamble ~7 +
DMA roundtrip ~2.3 + g phase), ~59.4us d-loop (PE-bound, 928 ns/iter,
all three engines 95-99% busy), ~8.4us tail (DVE reduce-bound),
~9.5us fixed framework teardown (249 unconditional semaphore resets).

Precision: float32r (FP22 in the PE) for g and layer-2, fp32 elsewhere;
  bf16 was measured to flip sdf signs in the randn-weight regime and is
  deliberately NOT used.
"""

import os
import sys

import numpy as np

for _p in ("/opt/trn_rl_repo", "/root/.axon_site/_ro/trn_rl_repo"):
    if os.path.isdir(_p) and _p not in sys.path:
        sys.path.append(_p)

from contextlib import ExitStack

from concourse import bacc, bass, masks, mybir, tile
from concourse.bass_utils import run_bass_kernel_spmd

F32 = mybir.dt.float32
F32R = mybir.dt.float32r
ALU = mybir.AluOpType
ACTF = mybir.ActivationFunctionType

IMG = 64
D = 64
HID = 128
BS = 4
NCORES = 8
PIX = IMG * IMG          # 4096 pixels per batch
PPC = PIX // 2           # 2048 pixels per core
NT = PPC // 128          # 16 pixel tiles per core
K63 = 63                 # depth pairs per tile

# engine split of the 2048 h' columns per depth step (GpSimd tensor_scalar
# measured ~17.6 ns/col on hw - software Q7 path - so the main loop stays on
# DVE+ACT only; GpSimd gets tail tensor_tensor work at ~2.3 ns/col instead).
# Psum chunks are <=512 wide (one bank) and >=256 (f32r 1 cycle/row needs
# moving size >= 256); chunks 0-2 are max-form (bias0), 3-4 relu (bias1).
DVE_N = 1280             # pixels 0..1279 on DVE (max-form)
ACT_N = 768              # pixels 1280..2047 on ACT (relu-form)
CHUNKS = [(0, 512), (512, 512), (1024, 256), (1280, 512), (1792, 256)]
CHUNK_TILES = [(0, 4), (4, 8), (8, 10), (10, 14), (14, 16)]

# consts tensor columns
C_NEGE = 0
C_EPOS = 1
C_B0 = 2
C_B1 = 3
C_DEPTH = 4              # cols 4..68 = depth[64]
C_W = 4 + D

_PROGRAM = None


def build_program():
    nc = bacc.Bacc(None, target_bir_lowering=False)
    vb = nc.declare_dram_parameter("vb", [3, HID], F32R, isOutput=False)
    qs = nc.declare_dram_parameter("qs", [3, PPC], F32R, isOutput=False)
    consts = nc.declare_dram_parameter("consts", [128, C_W], F32, isOutput=False)
    w2w = nc.declare_dram_parameter("w2w", [HID, 127], F32R, isOutput=False)
    out_o = nc.declare_dram_parameter("out", [128, 3 * NT], F32, isOutput=True)

    with tile.TileContext(nc) as tc, ExitStack() as ctx:
        const = ctx.enter_context(tc.tile_pool(name="const", bufs=1))
        gpool = ctx.enter_context(tc.tile_pool(name="g", bufs=1))
        hpool = ctx.enter_context(tc.tile_pool(name="hp", bufs=6))
        spool = ctx.enter_context(tc.tile_pool(name="s", bufs=1))
        post = ctx.enter_context(tc.tile_pool(name="post", bufs=1))
        pss = ctx.enter_context(tc.tile_pool(name="pss", bufs=1, space="PSUM"))
        pst = ctx.enter_context(tc.tile_pool(name="pst", bufs=2, space="PSUM"))

        # ---- input loads: vb at the head of the gpsimd queue (it can
        # enqueue ~0.4us after the preamble vs ~6.8us on sync, whose first
        # slot is delayed by a preamble drain); qs alone on sync ----
        t_vb = const.tile([3, HID], F32R, name="t_vb", tag="t_vb")
        nc.gpsimd.dma_start(t_vb[:], vb[:])
        t_qs = const.tile([3, PPC], F32R, name="t_qs", tag="t_qs")
        nc.sync.dma_start(t_qs[:], qs[:])
        # PE p-state warmup: dummy matmuls on a DVE-memset scratch tile while
        # the input DMAs are in flight (the PE clock ramps 0.65 -> 2.4 GHz
        # after ~3us of continuous work; without this the g matmuls and the
        # first loop iterations run up to ~2.4x slow)
        warm = const.tile([128, 256], F32, name="warm", tag="warm")
        nc.vector.memset(warm[:], 0.0)
        pwarm = pss.tile([64, 256], F32, name="pwarm", tag="pwarm")
        for _ in range(3):
            nc.tensor.matmul(
                pwarm[:], warm[:, 0:64], warm[:], start=True, stop=True)

        t_c = const.tile([128, C_W], F32, name="t_c", tag="t_c")
        nc.gpsimd.dma_start(t_c[:], consts[:])
        # sliding-window one-hot W2 (host-built): col 63 holds W2, rest
        # zero; slice [:, 63-d : 127-d] puts W2 in stationary column d ->
        # matmul accumulates s[d, :] into psum partition d.
        w2win = const.tile([HID, 127], F32R, name="w2win", tag="w2win")
        nc.gpsimd.dma_start(w2win[:], w2w[:])

        # identity for the tail transposes (gpsimd ops, needed only late)
        ident = const.tile([64, 64], F32)
        masks.make_identity(nc, ident[:])

        t_nege = t_c[:, C_NEGE:C_NEGE + 1]
        t_epos = t_c[:, C_EPOS:C_EPOS + 1]
        t_b0 = t_c[:, C_B0:C_B0 + 1]
        t_b1 = t_c[:, C_B1:C_B1 + 1]
        t_depth = t_c[:, C_DEPTH:C_DEPTH + D]

        # zm1[k] = depth[k] - 100 (first-crossing depth extraction); the d1
        # product reads it through a stride-0 broadcast AP over the tile axis
        zm1 = const.tile([128, K63], F32, name="zm1", tag="zm1")
        nc.vector.tensor_scalar(zm1[:], t_depth[:, 0:K63], -100.0, None, op0=ALU.add)

        # ---- g = Q @ V  (float32r matmul; one-time) ----
        # psum slots shared (by tag) with the layer-2 chunk tiles: the g
        # phase finishes before the d-loop's accumulators are first used.
        g_sb = gpool.tile([HID, PPC], F32)
        for k in range(4):
            pg = pss.tile([HID, 512], F32, name=f"pg{k}", tag=f"pss{k}")
            nc.tensor.matmul(
                pg[:], t_vb[:], t_qs[:, k * 512:(k + 1) * 512],
                start=True, stop=True,
            )
            dst = g_sb[:, k * 512:(k + 1) * 512]
            if k % 2 == 0:
                nc.vector.tensor_copy(dst, pg[:])
            else:
                nc.scalar.copy(dst, pg[:])

        # ---- main d-loop: h' then layer-2 (accumulating masked matmuls) ----
        ps_chunks = [
            pss.tile([D, n], F32, name=f"ps_chunk{k}", tag=f"pss{k}")
            for k, (off, n) in enumerate(CHUNKS)
        ]
        for d in range(D):
            hp = hpool.tile([HID, PPC], F32R, tag="hp")
            dep_d = t_depth[:, d:d + 1]
            if d == 0:
                # split along the g-copy chunks so the first layer-2
                # matmuls start before the last g copy lands
                for o0, o1 in ((0, 512), (512, 1024), (1024, DVE_N)):
                    nc.vector.tensor_scalar(
                        hp[:, o0:o1], g_sb[:, o0:o1],
                        dep_d, t_nege, op0=ALU.mult, op1=ALU.max,
                    )
            else:
                nc.vector.tensor_scalar(
                    hp[:, 0:DVE_N], g_sb[:, 0:DVE_N],
                    dep_d, t_nege, op0=ALU.mult, op1=ALU.max,
                )
            nc.scalar.activation(
                hp[:, DVE_N:PPC], g_sb[:, DVE_N:PPC], ACTF.Relu,
                bias=t_epos, scale=dep_d,
            )
            w2slice = w2win[:, 63 - d:127 - d]
            for k, (off, n) in enumerate(CHUNKS):
                nc.tensor.matmul(
                    ps_chunks[k][:], w2slice, hp[:, off:off + n],
                    start=(d == 0), stop=(d == D - 1),
                    skip_group_check=True,
                )

        # ---- tail: per-chunk copy, transpose, tanh; then postprocess ----
        s_sb = spool.tile([D, PPC], F32)
        sdf = post.tile([128, NT * D], F32)
        pos = post.tile([128, NT * D], F32)
        zc = post.tile([128, NT * K63], F32)
        out_sb = post.tile([128, 3 * NT], F32)
        sdf3 = sdf[:].rearrange("p (t d) -> p t d", d=D)
        pos3 = pos[:].rearrange("p (t d) -> p t d", d=D)
        zc3 = zc[:].rearrange("p (t k) -> p t k", k=K63)
        zm1_1 = zm1[:].rearrange("p (o k) -> p o k", o=1)
        for k, (off, n) in enumerate(CHUNKS):
            if k % 2 == 0:
                nc.scalar.copy(s_sb[:, off:off + n], ps_chunks[k][:])
            else:
                nc.vector.tensor_copy(s_sb[:, off:off + n], ps_chunks[k][:])
            t0, t1 = CHUNK_TILES[k]
            # transposes share one psum tile per chunk -> single batched tanh
            # (chunks 0-2 are all max-form tiles, chunks 3-4 relu-form, so
            # the per-partition tanh bias is uniform within a chunk)
            pt = pst.tile([128, (t1 - t0) * D], F32, name=f"pt{k}", tag="pt", bufs=2)
            for j, i in enumerate(range(t0, t1)):
                nc.tensor.transpose(
                    pt[:, j * D:(j + 1) * D], s_sb[:, i * 128:(i + 1) * 128], ident[:])
            b_ap = t_b0 if k < 3 else t_b1
            nc.scalar.activation(
                sdf[:, t0 * D:t1 * D], pt[:, 0:(t1 - t0) * D], ACTF.Tanh,
                bias=b_ap, scale=1.0,
            )

        # postprocess in 2 tile groups (pipelined); outputs are packed
        # GROUP-major -- group g owns out_sb[:, g*24:(g+1)*24] as
        # [s1(8) | s2(8) | d1r(8)] -- so each group's half can DMA out as
        # soon as its own reduces finish, overlapping the other group
        for t0, t1 in ((0, 8), (8, 16)):
            ts_ = slice(t0, t1)
            nc.vector.tensor_scalar(
                pos[:, t0 * D:t1 * D],
                sdf[:, t0 * D:t1 * D], 0.0, None, op0=ALU.is_gt)
            nc.vector.scalar_tensor_tensor(
                zc3[:, ts_, :], pos3[:, ts_, 1:D], 0.5, pos3[:, ts_, 0:K63],
                op0=ALU.is_lt, op1=ALU.mult,
            )
            nw = (t1 - t0) * K63
            # s2/d1 products on GpSimd (run in parallel); reduces are DVE-only
            for qi, (in1, red_op, oc, eng_m) in enumerate((
                (sdf3[:, ts_, 0:K63], ALU.add, 0, nc.vector),
                (sdf3[:, ts_, 1:D], ALU.add, 1, nc.gpsimd),
                (zm1_1.broadcast_to([128, t1 - t0, K63]), ALU.min, 2, nc.gpsimd),
            )):
                tmp = post.tile([128, 8 * K63], F32, tag=f"ppt{qi}", bufs=2)
                tmp3 = tmp[:, 0:nw].rearrange("p (t k) -> p t k", k=K63)
                eng_m.tensor_tensor(tmp3, zc3[:, ts_, :], in1, op=ALU.mult)
                g24 = (t0 // 8) * 24
                nc.vector.tensor_reduce(
                    out_sb[:, g24 + oc * 8:g24 + oc * 8 + 8], tmp3,
                    axis=mybir.AxisListType.X, op=red_op)
            # per-group output DMA on the sync ring (lowest completion
            # latency); group 1's half transfers while group 2 reduces
            g24 = (t0 // 8) * 24
            nc.sync.dma_start(
                out_o[:, g24:g24 + 24], out_sb[:, g24:g24 + 24])

    nc.finalize()
    return nc


def host_prep(z_shape, z_extr, W1, b1, W2, b2):
    """Per-core input maps. All small math mirrors the reference in
    float64 (deviations ~1e-7, far inside the sdf sign margins)."""
    f32 = np.float32
    z_shape = np.asarray(z_shape, f32)
    z_extr = np.asarray(z_extr, f32)
    W1 = np.asarray(W1, f32)
    b1 = np.asarray(b1, f32)
    W2 = np.asarray(W2, f32)
    b2 = np.asarray(b2, f32)

    f = 70.0 * (IMG / 64.0)
    cc = IMG / 2.0 - 0.5
    Km = np.array([[f, 0, cc], [0, f, cc], [0, 0, 1]], np.float64)
    K_inv = np.linalg.inv(Km)
    t = np.array([0.0, 0.0, 2.5])

    # mirror the reference's f32 double-reciprocal
    s_obj32 = (1.0 / z_extr[:, 0]).astype(f32)
    s_inv32 = (1.0 / s_obj32).astype(f32)
    s_obj = s_obj32.astype(np.float64)
    s_inv = s_inv32.astype(np.float64)
    t_obj = z_extr[:, 1:4].astype(np.float64)
    alpha = z_extr[:, 4].astype(np.float64)

    a = np.pi * alpha
    ca, sa = np.cos(a), np.sin(a)
    R_obj = np.zeros((BS, 3, 3))
    R_obj[:, 0, 0] = ca
    R_obj[:, 0, 1] = -sa
    R_obj[:, 1, 0] = sa
    R_obj[:, 1, 1] = ca
    R_obj[:, 2, 2] = 1.0

    corners = np.array(
        [[1, 1, 1], [1, 1, -1], [1, -1, 1], [1, -1, -1],
         [-1, 1, 1], [-1, 1, -1], [-1, -1, 1], [-1, -1, -1], [0, 0, 0]],
        np.float64,
    )
    R_obj_inv = np.linalg.inv(R_obj)
    # z-component of K @ (R_t^-1 (R_obj_inv (s_inv * corner) + t_obj) + t)
    zcr = np.einsum("bij,aj->bai", R_obj_inv, corners)[:, :, 2] * s_inv[:, None]
    bb_depth = zcr.mean(axis=1) + t_obj[:, 2] + 2.5      # (BS,)

    zs = np.linspace(-1.0, 1.0, D)
    depth_bd = (zs[None, :] * s_inv[:, None] + bb_depth[:, None]).astype(f32)

    M = s_obj[:, None, None] * R_obj
    c_b = np.einsum("bij,bj->bi", M, -(t[None, :] + t_obj))
    V = np.einsum("bij,ih->bjh", M, W1[:3].astype(np.float64))   # (BS,3,H)
    e = (
        np.einsum("bi,ih->bh", c_b, W1[:3].astype(np.float64))
        + z_shape.astype(np.float64) @ W1[3:].astype(np.float64)
        + b1.astype(np.float64)
    )
    e32 = e.astype(f32)
    s0 = (e32.astype(np.float64) @ W2.astype(np.float64) + b2.astype(np.float64))
    s0 = s0.astype(f32)                                  # (BS,1)

    xs = np.linspace(0.0, IMG - 1.0, IMG)
    Xg, Yg = np.meshgrid(xs, xs)
    p3 = np.stack([Xg.reshape(-1), Yg.reshape(-1), np.ones(PIX)], -1)
    q = p3 @ K_inv.T                                     # (PIX, 3)

    w2win_host = np.zeros((HID, 127), f32)
    w2win_host[:, 63] = W2[:, 0]
    in_maps = []
    lam_c = []
    for c in range(NCORES):
        b, half = c // 2, c % 2
        qs_c = q[half * PPC:(half + 1) * PPC].T.astype(f32)        # (3, PPC)
        consts = np.zeros((128, C_W), f32)
        consts[:, C_NEGE] = -e32[b]
        consts[:, C_EPOS] = e32[b]
        consts[:, C_B0] = s0[b, 0]
        consts[:, C_B1] = b2[0]
        consts[:, C_DEPTH:C_DEPTH + D] = depth_bd[b][None, :]
        in_maps.append({
            "vb": np.ascontiguousarray(V[b].astype(f32)),
            "qs": np.ascontiguousarray(qs_c),
            "consts": consts,
            "w2w": w2win_host,
        })
        lam_c.append(np.float32(depth_bd[b][1] - depth_bd[b][0]))
    return in_maps, lam_c


def _assemble(results, lam_c):
    """Unpack s1/s2/d1r per core and run the final interpolation on host in
    fp32, mirroring the reference arithmetic."""
    f32 = np.float32
    dp_full = np.zeros((BS, PIX), f32)
    occ_full = np.zeros((BS, PIX), f32)
    eps = f32(1e-6)
    hundred = f32(100.0)
    for c in range(NCORES):
        b, half = c // 2, c % 2
        out = np.asarray(results[c]["out"])               # [128, 48]
        # group-major packing: group g at cols [g*24:(g+1)*24] = s1|s2|d1r
        s1 = np.concatenate([out[:, 0:8], out[:, 24:32]], axis=1)
        s2 = np.concatenate([out[:, 8:16], out[:, 32:40]], axis=1)
        d1r = np.concatenate([out[:, 16:24], out[:, 40:48]], axis=1)
        occ = (d1r <= f32(-50.0)).astype(f32)
        d1 = (d1r + hundred).astype(f32)
        den = (s2 - s1 - eps).astype(f32)
        dp = (occ * (d1 - s1 / den * lam_c[c])).astype(f32)
        sl = slice(half * PPC, (half + 1) * PPC)
        dp_full[b, sl] = dp.T.ravel()
        occ_full[b, sl] = occ.T.ravel()
    return (
        dp_full.reshape(BS, IMG, IMG, 1),
        occ_full.reshape(BS, IMG, IMG, 1),
    )


def get_program():
    global _PROGRAM
    if _PROGRAM is None:
        _PROGRAM = build_program()
    return _PROGRAM


def kernel(z_shape, z_extr, W1, b1, W2, b2, **run_kwargs):
    nc = get_program()
    in_maps, lam_c = host_prep(z_shape, z_extr, W1, b1, W2, b2)
    res = run_bass_kernel_spmd(nc, in_maps, core_ids=list(range(NCORES)), **run_kwargs)
    out = _assemble(res.results, lam_c)
    if run_kwargs:
        return out, res
    return out
